# revision 10
# baseline (speedup 1.0000x reference)
"""Trainium2 Bass kernel for BitwiseTasNetBlock (v3).

Data-parallel over batch: 8 cores x 2 batch items, activations bf16.

Structure per layer (Y1(i) already in ybuf[i%2], produced by the fused
F(i-1) matmul or by the layer-0 input conv A0):
  - dconv run-ahead: a few PE diagonal-matmul units drained raw
    (Act.Copy) so the PE works through the stats1 collective
  - sync1: AllReduce of (sum, sumsq) -> BN1 scale s1 / shift t1
  - B: depthwise dilated conv per unit on PE (diag matmuls + fused
    BN1+PReLU ACT drain) or DVE (ts+stt+stt chain with s1 folded into
    tap scalars + ACT PReLU), writing P2 in place of Y1 (reverse
    chunk-pair order)
  - sync2: AllReduce -> BN2
  - F (i<3): fused conv2(i)+conv1(i+1): lhsT = (W1(i+1)@W2(i)) *
    diag(s2) (host-precomputed integer product, exact in bf16; s2
    column-scale applied on chip), bias folded into the drain bias
    b1p(i+1) = b1(i+1) + W1(i+1)@b2(i) [host] + Wf@t2 [tiny matmul].
    Drains (ACT PReLU) write Y1(i+1) to the other ybuf.
    C (i==3): w2*s2 matmuls + bias3 + residual (DVE) -> DMA out f32.

Stats use no bn_stats: every ACT drain emits sum(y) via accum_out for
free; sum(y^2) is one extra op per unit (DVE stt-accum or ACT Square
with accum, split to balance engines).
"""
import sys

sys.path.insert(0, "/opt/trn_rl_repo")
import numpy as np

L, CB, D, KTAP = 4, 256, 512, 3
B, T = 16, 4096
EPS = 1e-5
NCORES = 8
BLOC = B // NCORES
CHUNK = 512
NCT = T // CHUNK
PAIR = 2 * CHUNK
NPAIR = NCT // 2
PAD = 16
DOFF = PAD
YCOLS = PAD + T
NGLOB = float(B * T)

# chunk pairs in reverse time order: (6,7),(4,5),(2,3),(0,1) as (c0,c1)
PAIRS = [(NCT - 2 * p - 2, NCT - 2 * p - 1) for p in range(NPAIR)]

# --- tuning knobs ---
# dconv units are (b, pair_idx, ct); DVE-mode units per layer by (b, pi):
DVE_PI = {0: [(1, 0), (1, 1), (1, 2), (0, 2)],
          1: [(1, 0), (1, 1), (1, 2), (0, 2)],
          2: [(1, 0), (1, 1), (1, 2), (0, 2)],
          3: [(1, 0), (1, 1), (1, 2), (0, 2)]}
# PE-mode units drained raw before sync1 (fixed up after):
DEFER_PI = [(0, 0), (0, 1)]
# sum(y) placement: ACT accum (cheap 287ns read) or DVE ts-accum; split
# to balance engines. sum(y^2) always DVE stt-accum.


def _sy_on_act(pi, sub):
    return (pi + sub) % 2 == 0


(V_B1C, V_BD, V_G1, V_BE1, V_G2, V_BE2, V_WSA, V_WS12, V_WS2,
 V_WD0, V_WD1, V_WD2) = range(12)
NVEC = 12

_cache = {}


def _build(a1_vals, a2_vals):
    import concourse.bass as bass
    import concourse.tile as tile
    from concourse import bacc, mybir

    f32 = mybir.dt.float32
    bf16 = mybir.dt.bfloat16
    Alu = mybir.AluOpType
    Act = mybir.ActivationFunctionType

    nc = bacc.Bacc(None, target_bir_lowering=False, debug=False, num_devices=NCORES)

    xin_d = nc.dram_tensor("xin", [BLOC, CB, T], bf16, kind="ExternalInput")
    w1t_d = nc.dram_tensor("w1t", [128, 2, 4, 128], bf16, kind="ExternalInput")
    wf_d = nc.dram_tensor("wf", [L - 1, 128, 4, 4, 128], bf16, kind="ExternalInput")
    w2t_d = nc.dram_tensor("w2t", [128, 4, 2, 128], bf16, kind="ExternalInput")
    wdg_d = nc.dram_tensor("wdg", [L, 128, 3, 4, 128], bf16, kind="ExternalInput")
    vecs_d = nc.dram_tensor("vecs", [128, L, NVEC, 4], f32, kind="ExternalInput")
    b2_d = nc.dram_tensor("b2v", [128, 2], f32, kind="ExternalInput")
    out_d = nc.dram_tensor("out", [BLOC, CB, T], f32, kind="ExternalOutput")

    with tile.TileContext(nc) as tc:
        with (
            tc.tile_pool(name="ybufp", bufs=1) as ybufp,
            tc.tile_pool(name="constp", bufs=1) as constp,
            tc.tile_pool(name="wp", bufs=2) as wp,
            tc.tile_pool(name="xstg", bufs=3) as xstgp,
            tc.tile_pool(name="zbp", bufs=2) as zbp,
            tc.tile_pool(name="sqp", bufs=3) as sqp,
            tc.tile_pool(name="rstg", bufs=3) as rstgp,
            tc.tile_pool(name="ostg", bufs=2) as ostgp,
            tc.tile_pool(name="vec", bufs=8) as vecp,
            tc.tile_pool(name="ps", bufs=4, space="PSUM") as psp,
            tc.tile_pool(name="dram", bufs=4, space="DRAM") as dramp,
        ):
            # ping-pong activation buffers: ybuf[s][ct]
            ybuf = [
                [ybufp.tile([128, BLOC, YCOLS], bf16, tag=f"ybuf{s}{ct}",
                            name=f"ybuf{s}{ct}") for ct in range(4)]
                for s in range(2)
            ]

            vecs_sb = constp.tile([128, L, NVEC, 4], f32)
            b2_sb = constp.tile([128, 2], f32)
            nc.scalar.dma_start(vecs_sb[:], vecs_d[:])
            nc.scalar.dma_start(b2_sb[:], b2_d[:])

            w1sb = constp.tile([128, 2, 4, 128], bf16, tag="w1sb")
            w2sb = constp.tile([128, 4, 2, 128], bf16, tag="w2sb")
            wfsb = [wp.tile([128, 4, 4, 128], bf16, tag="wfsb", name=f"wfsb{i}")
                    for i in range(2)]
            wdsb = [wp.tile([128, 3, 4, 128], bf16, tag="wdsb", name=f"wdsb{i}")
                    for i in range(2)]
            nc.scalar.dma_start(w1sb[:], w1t_d[:])
            nc.scalar.dma_start(wdsb[0][:], wdg_d[0])
            nc.scalar.dma_start(wfsb[0][:], wf_d[0])

            # startup alignment dummy AllReduce
            aln_in = dramp.tile([128, 1], f32, tag="alnin")
            aln_out = dramp.tile([128, 1], f32, tag="alnout")
            alnsb = constp.tile([128, 1], f32, tag="alnsb")
            nc.vector.memset(alnsb[:], 0.0)
            nc.sync.dma_start(aln_in[:], alnsb[:])
            nc.gpsimd.collective_compute(
                "AllReduce", Alu.add,
                replica_groups=[list(range(NCORES))],
                ins=[aln_in[:].opt()], outs=[aln_out[:].opt()],
            )

            for s in range(2):
                for ct in range(4):
                    for b in range(BLOC):
                        nc.vector.memset(ybuf[s][ct][:, b, 0:PAD], 0.0)
            epsc = constp.tile([128, 1], f32, tag="epsc")
            nc.vector.memset(epsc[:], EPS)

            # per-layer drain bias (b1 + folded conv2 bias of prev layer)
            b1p = [vecp.tile([128, 4], f32, tag="b1p", name=f"b1p{i}")
                   for i in range(L)]
            nc.vector.tensor_copy(b1p[0][:], vecs_sb[:, 0, V_B1C, :])

            # stats accumulators per layer: [128, 4(sub), 8(b*4+pi)]
            st1 = []
            st2 = []
            for i in range(L):
                st1.append((
                    vecp.tile([128, 4, 8], f32, tag="asy", name=f"asy{i}"),
                    vecp.tile([128, 4, 8], f32, tag="asq", name=f"asq{i}"),
                ))
                st2.append((
                    vecp.tile([128, 4, 8], f32, tag="bsy", name=f"bsy{i}"),
                    vecp.tile([128, 4, 8], f32, tag="bsq", name=f"bsq{i}"),
                ))

            def sq_accum(y_sl, sy_sl, sq_sl, pi, sub):
                """sum(y) (if sy_sl given) and sum(y^2) for one [1024]
                unit on the vector engine."""
                if sy_sl is not None:
                    scr0 = sqp.tile([128, PAIR], bf16, tag="sy_scr")
                    nc.vector.tensor_scalar(
                        scr0[:], y_sl, 1.0, 0.0, op0=Alu.mult, op1=Alu.add,
                        accum_out=sy_sl)
                scr = sqp.tile([128, PAIR], bf16, tag="sqv")
                nc.vector.scalar_tensor_tensor(
                    scr[:], y_sl, 1.0, y_sl, op0=Alu.mult, op1=Alu.mult,
                    accum_out=sq_sl)

            def stat_sync(i, st, g_idx, be_idx):
                """Reduce accumulators, AllReduce, produce s4/t4 [128,4]."""
                sy, sq = st
                csb = vecp.tile([128, 8], f32, tag="csb")
                nc.vector.tensor_reduce(
                    csb[:, 0:4], sy[:], axis=mybir.AxisListType.X, op=Alu.add)
                nc.vector.tensor_reduce(
                    csb[:, 4:8], sq[:], axis=mybir.AxisListType.X, op=Alu.add)
                cin = dramp.tile([128, 8], f32, tag="cin")
                cout = dramp.tile([128, 8], f32, tag="cout")
                nc.sync.dma_start(cin[:], csb[:])
                nc.gpsimd.collective_compute(
                    "AllReduce", Alu.add,
                    replica_groups=[list(range(NCORES))],
                    ins=[cin[:].opt()], outs=[cout[:].opt()],
                )
                gsb = vecp.tile([128, 8], f32, tag="gsb")
                nc.sync.dma_start(gsb[:], cout[:])
                mean4 = vecp.tile([128, 4], f32, tag="mean4")
                nc.vector.tensor_scalar(
                    mean4[:], gsb[:, 0:4], 1.0 / NGLOB, None, op0=Alu.mult)
                var4 = vecp.tile([128, 4], f32, tag="var4")
                nc.vector.tensor_scalar(
                    var4[:], gsb[:, 4:8], 1.0 / NGLOB, None, op0=Alu.mult)
                m24 = vecp.tile([128, 4], f32, tag="m24")
                nc.vector.tensor_mul(m24[:], mean4[:], mean4[:])
                nc.vector.tensor_sub(var4[:], var4[:], m24[:])
                std4 = vecp.tile([128, 4], f32, tag="std4")
                nc.scalar.activation(std4[:], var4[:], Act.Sqrt, bias=epsc[:],
                                     scale=1.0)
                rstd4 = vecp.tile([128, 4], f32, tag="rstd4")
                nc.vector.reciprocal(rstd4[:], std4[:])
                s4 = vecp.tile([128, 4], f32, tag="s4")
                nc.vector.tensor_mul(s4[:], rstd4[:], vecs_sb[:, i, g_idx, :])
                t4 = vecp.tile([128, 4], f32, tag="t4")
                nc.vector.tensor_mul(t4[:], mean4[:], s4[:])
                nc.vector.tensor_sub(t4[:], vecs_sb[:, i, be_idx, :], t4[:])
                return s4, t4

            # ---- A0: layer-0 input conv ----
            a1_0 = float(a1_vals[0])
            for b in range(BLOC):
                for pi, (c0, c1) in enumerate(PAIRS):
                    xst = []
                    for c in (c0, c1):
                        xs = xstgp.tile([128, PAIR], bf16, tag="xs",
                                        name=f"xs_{b}_{c}")
                        for kt in range(2):
                            eng = nc.sync if kt == 0 else nc.scalar
                            eng.dma_start(
                                xs[:, kt * CHUNK:(kt + 1) * CHUNK],
                                xin_d[b, 128 * kt:128 * (kt + 1),
                                      CHUNK * c:CHUNK * (c + 1)])
                        xst.append(xs)
                    for mt in range(4):
                        ps = psp.tile([128, PAIR], f32, tag="ps")
                        for kt in range(2):
                            for h in range(2):
                                nc.tensor.matmul(
                                    ps[:, h * CHUNK:(h + 1) * CHUNK],
                                    w1sb[:, kt, mt, :],
                                    xst[h][:, kt * CHUNK:(kt + 1) * CHUNK],
                                    start=(kt == 0), stop=(kt == 1))
                        ysl = ybuf[0][mt][:, b, DOFF + CHUNK * c0:
                                          DOFF + CHUNK * c0 + PAIR]
                        sy_sl = st1[0][0][:, mt, b * 4 + pi:b * 4 + pi + 1]
                        on_act = _sy_on_act(pi, mt)
                        nc.scalar.activation(
                            ysl, ps[:], Act.Prelu, bias=b1p[0][:, mt:mt + 1],
                            scale=1.0, alpha=a1_0,
                            accum_out=sy_sl if on_act else None)
                        sq_accum(ysl, None if on_act else sy_sl,
                                 st1[0][1][:, mt, b * 4 + pi:b * 4 + pi + 1],
                                 pi, mt)

            # ---- layer loop ----
            for i in range(L):
                dil = 2 ** i
                a2i = float(a2_vals[i])
                wsl = i % 2
                ybi = ybuf[i % 2]
                wdr = wdsb[wsl]
                if i + 1 < L:
                    nsl = (i + 1) % 2
                    nc.scalar.dma_start(wdsb[nsl][:], wdg_d[i + 1])
                    if i + 1 <= L - 2:
                        nc.scalar.dma_start(wfsb[nsl][:], wf_d[i + 1])
                if i == L - 2:
                    nc.scalar.dma_start(w2sb[:], w2t_d[:])

                def emit_dconv_mm(b, c0, c1, ct, _dil=dil, _wdr=wdr, _ybi=ybi,
                                  _i=i):
                    ps = psp.tile([128, PAIR], f32, tag="ps",
                                  name=f"psd_{_i}_{b}_{c0}_{ct}")
                    for j in range(KTAP):
                        for h, c in ((0, c0), (1, c1)):
                            base = DOFF + CHUNK * c
                            off = base - (2 - j) * _dil
                            nc.tensor.matmul(
                                ps[:, h * CHUNK:(h + 1) * CHUNK],
                                _wdr[:, j, ct, :],
                                _ybi[ct][:, b, off:off + CHUNK],
                                start=(j == 0), stop=(j == KTAP - 1))
                    return ps

                # run-ahead PE units, raw drains (no s1 dependency)
                deferred = []
                for (b, pi) in DEFER_PI:
                    c0, c1 = PAIRS[pi]
                    for ct in range(4):
                        ps = emit_dconv_mm(b, c0, c1, ct)
                        dst = ybi[ct][:, b, DOFF + CHUNK * c0:
                                      DOFF + CHUNK * c0 + PAIR]
                        nc.scalar.activation(dst, ps[:], Act.Copy)
                        deferred.append((b, pi, c0, ct))

                # ---- sync1 ----
                s1, t1 = stat_sync(i, st1[i], V_G1, V_BE1)
                b2a4 = vecp.tile([128, 4], f32, tag="b2a4")
                b2b4 = vecp.tile([128, 4], f32, tag="b2b4")
                b2c4 = vecp.tile([128, 4], f32, tag="b2c4")
                for dst4, widx in ((b2a4, V_WSA), (b2b4, V_WS12), (b2c4, V_WS2)):
                    nc.vector.tensor_mul(dst4[:], t1[:], vecs_sb[:, i, widx, :])
                    nc.vector.tensor_add(dst4[:], dst4[:], vecs_sb[:, i, V_BD, :])
                s1w = []
                for j in range(KTAP):
                    sw = vecp.tile([128, 4], f32, tag=f"s1w{j}", name=f"s1w{j}_{i}")
                    nc.vector.tensor_mul(sw[:], s1[:], vecs_sb[:, i, V_WD0 + j, :])
                    s1w.append(sw)

                # deferred fixups: in-place PReLU with BN1 scale/shift + stats
                for (b, pi, c0, ct) in deferred:
                    dst = ybi[ct][:, b, DOFF + CHUNK * c0:DOFF + CHUNK * c0 + PAIR]
                    sy_sl = st2[i][0][:, ct, b * 4 + pi:b * 4 + pi + 1]
                    on_act = _sy_on_act(pi, ct)
                    nc.scalar.activation(
                        dst, dst, Act.Prelu, bias=b2a4[:, ct:ct + 1],
                        scale=s1[:, ct:ct + 1], alpha=a2i,
                        accum_out=sy_sl if on_act else None)
                    sq_accum(dst, None if on_act else sy_sl,
                             st2[i][1][:, ct, b * 4 + pi:b * 4 + pi + 1],
                             pi, ct)

                # ---- B: remaining dconv units ----
                for b in range(BLOC):
                    for pi, (c0, c1) in enumerate(PAIRS):
                        if (b, pi) in DEFER_PI:
                            continue
                        for ct in range(4):
                            dst = ybi[ct][:, b, DOFF + CHUNK * c0:
                                          DOFF + CHUNK * c0 + PAIR]
                            sy_sl = st2[i][0][:, ct, b * 4 + pi:b * 4 + pi + 1]
                            on_act = _sy_on_act(pi, ct)
                            acc_sl = sy_sl if on_act else None
                            if (b, pi) in DVE_PI[i]:
                                zb = zbp.tile([128, PAIR], bf16, tag="zb")
                                base = DOFF + CHUNK * c0
                                nc.vector.tensor_scalar(
                                    zb[:], ybi[ct][:, b, base - 2 * dil:
                                                   base - 2 * dil + PAIR],
                                    s1w[0][:, ct:ct + 1], b2a4[:, ct:ct + 1],
                                    op0=Alu.mult, op1=Alu.add)
                                nc.vector.scalar_tensor_tensor(
                                    zb[:], ybi[ct][:, b, base - dil:
                                                   base - dil + PAIR],
                                    s1w[1][:, ct:ct + 1], zb[:],
                                    op0=Alu.mult, op1=Alu.add)
                                nc.vector.scalar_tensor_tensor(
                                    zb[:], ybi[ct][:, b, base:base + PAIR],
                                    s1w[2][:, ct:ct + 1], zb[:],
                                    op0=Alu.mult, op1=Alu.add)
                                nc.scalar.activation(dst, zb[:], Act.Prelu,
                                                     alpha=a2i, accum_out=acc_sl)
                            else:
                                ps = emit_dconv_mm(b, c0, c1, ct)
                                nc.scalar.activation(
                                    dst, ps[:], Act.Prelu,
                                    bias=b2a4[:, ct:ct + 1],
                                    scale=s1[:, ct:ct + 1], alpha=a2i,
                                    accum_out=acc_sl)
                                if c0 == 0:
                                    po = DOFF
                                    nc.scalar.activation(
                                        ybi[ct][:, b, po:po + dil], ps[:, 0:dil],
                                        Act.Prelu, bias=b2c4[:, ct:ct + 1],
                                        scale=s1[:, ct:ct + 1], alpha=a2i)
                                    nc.scalar.activation(
                                        ybi[ct][:, b, po + dil:po + 2 * dil],
                                        ps[:, dil:2 * dil], Act.Prelu,
                                        bias=b2b4[:, ct:ct + 1],
                                        scale=s1[:, ct:ct + 1], alpha=a2i)
                            sq_accum(dst, None if on_act else sy_sl,
                                     st2[i][1][:, ct, b * 4 + pi:b * 4 + pi + 1],
                                     pi, ct)
                # note: the c0==0 boundary rewrites (2*dil of 65536 cols per
                # channel) slightly perturb sum(y)/sum(y^2) vs exact; the
                # relative stats error is ~dil/65536 -- far below bf16 noise.

                # ---- sync2 + weight folding ----
                s2, t2 = stat_sync(i, st2[i], V_G2, V_BE2)
                if i < L - 1:
                    a1n = float(a1_vals[i + 1])
                    wfr = wfsb[wsl]
                    wfs = wp.tile([128, 4, 4, 128], bf16, tag="wfs")
                    for kt in range(4):
                        nc.vector.tensor_scalar(
                            wfs[:, kt, :, :], wfr[:, kt, :, :],
                            s2[:, kt:kt + 1], None, op0=Alu.mult)
                    # b1p[i+1] = b1c[i+1] + Wf_raw @ t2
                    t2bf = vecp.tile([128, 4, 2], bf16, tag="t2bf")
                    nc.vector.tensor_copy(t2bf[:, :, 0], t2[:])
                    nc.vector.tensor_copy(t2bf[:, :, 1], t2[:])
                    psf = psp.tile([128, PAIR], f32, tag="ps", name=f"psf_{i}")
                    for mt in range(4):
                        for kt in range(4):
                            nc.tensor.matmul(
                                psf[:, 2 * mt:2 * mt + 2],
                                wfr[:, kt, mt, :],
                                t2bf[:, kt, :],
                                start=(kt == 0), stop=(kt == 3))
                        nc.scalar.activation(
                            b1p[i + 1][:, mt:mt + 1], psf[:, 2 * mt:2 * mt + 1],
                            Act.Identity,
                            bias=vecs_sb[:, i + 1, V_B1C, mt:mt + 1], scale=1.0)
                    # ---- F: fused conv2(i) + conv1(i+1) ----
                    ybn = ybuf[(i + 1) % 2]
                    for b in range(BLOC):
                        for pi, (c0, c1) in enumerate(PAIRS):
                            po0 = DOFF + CHUNK * c0
                            for mt in range(4):
                                ps = psp.tile([128, PAIR], f32, tag="ps")
                                for kt in range(4):
                                    for h in range(2):
                                        nc.tensor.matmul(
                                            ps[:, h * CHUNK:(h + 1) * CHUNK],
                                            wfs[:, kt, mt, :],
                                            ybi[kt][:, b, po0 + h * CHUNK:
                                                    po0 + (h + 1) * CHUNK],
                                            start=(kt == 0), stop=(kt == 3))
                                ysl = ybn[mt][:, b, po0:po0 + PAIR]
                                sy_sl = st1[i + 1][0][
                                    :, mt, b * 4 + pi:b * 4 + pi + 1]
                                on_act = _sy_on_act(pi, mt)
                                nc.scalar.activation(
                                    ysl, ps[:], Act.Prelu,
                                    bias=b1p[i + 1][:, mt:mt + 1], scale=1.0,
                                    alpha=a1n,
                                    accum_out=sy_sl if on_act else None)
                                sq_accum(ysl, None if on_act else sy_sl,
                                         st1[i + 1][1][:, mt,
                                                       b * 4 + pi:b * 4 + pi + 1],
                                         pi, mt)
                else:
                    # ---- C (last layer): w2*s2 + bias3 + residual ----
                    w2r = wp.tile([128, 4, 2, 128], bf16, tag="w2r")
                    for kt in range(4):
                        nc.vector.tensor_scalar(
                            w2r[:, kt, :, :], w2sb[:, kt, :, :],
                            s2[:, kt:kt + 1], None, op0=Alu.mult)
                    rec4 = vecp.tile([128, 4], f32, tag="rec4")
                    nc.vector.reciprocal(rec4[:], s2[:])
                    r24 = vecp.tile([128, 4, 2], bf16, tag="r24")
                    nc.vector.tensor_mul(r24[:, :, 0], t2[:], rec4[:])
                    nc.vector.tensor_mul(r24[:, :, 1], t2[:], rec4[:])
                    psb = psp.tile([128, PAIR], f32, tag="ps", name="psb3")
                    b3f = vecp.tile([128, 2], f32, tag="b3f")
                    for mt in range(2):
                        for kt in range(4):
                            nc.tensor.matmul(
                                psb[:, 2 * mt:2 * mt + 2],
                                w2r[:, kt, mt, :],
                                r24[:, kt, :],
                                start=(kt == 0), stop=(kt == 3))
                        nc.scalar.activation(
                            b3f[:, mt:mt + 1], psb[:, 2 * mt:2 * mt + 1],
                            Act.Identity, bias=b2_sb[:, mt:mt + 1], scale=1.0)
                    for b in range(BLOC):
                        for c in range(NCT - 1, -1, -1):
                            po = DOFF + CHUNK * c
                            rt = rstgp.tile([128, PAIR], bf16, tag="rt")
                            for mt in range(2):
                                nc.scalar.dma_start(
                                    rt[:, mt * CHUNK:(mt + 1) * CHUNK],
                                    xin_d[b, 128 * mt:128 * (mt + 1),
                                          CHUNK * c:CHUNK * (c + 1)])
                            ps = psp.tile([128, PAIR], f32, tag="ps")
                            for mt in range(2):
                                for kt in range(4):
                                    nc.tensor.matmul(
                                        ps[:, mt * CHUNK:(mt + 1) * CHUNK],
                                        w2r[:, kt, mt, :],
                                        ybi[kt][:, b, po:po + CHUNK],
                                        start=(kt == 0), stop=(kt == 3))
                            ot = ostgp.tile([128, PAIR], f32, tag="ot")
                            for mt in range(2):
                                nc.vector.scalar_tensor_tensor(
                                    ot[:, mt * CHUNK:(mt + 1) * CHUNK],
                                    ps[:, mt * CHUNK:(mt + 1) * CHUNK],
                                    b3f[:, mt:mt + 1],
                                    rt[:, mt * CHUNK:(mt + 1) * CHUNK],
                                    op0=Alu.add, op1=Alu.add)
                                eng = nc.sync if c % 2 == 0 else nc.scalar
                                eng.dma_start(
                                    out_d[b, 128 * mt:128 * (mt + 1),
                                          CHUNK * c:CHUNK * (c + 1)],
                                    ot[:, mt * CHUNK:(mt + 1) * CHUNK])

    nc.compile()
    return nc


def _prep_inputs(x, w1, b1, a1, g1, be1, wd, bd, a2, g2, be2, w2, b2):
    """Host-side packing. Binarized weights; fused W1(i+1)@W2(i) products
    (integer-valued, exact in bf16)."""
    import ml_dtypes

    bf = ml_dtypes.bfloat16
    w1b = np.sign(w1[..., 0]).astype(np.float32)  # [L, D, CB]
    wdb = np.sign(wd[..., 0, :]) if wd.ndim == 4 else np.sign(wd[:, :, 0, :])
    wdb = wdb.astype(np.float32)  # [L, D, K]
    w2b = np.sign(w2[..., 0]).astype(np.float32)  # [L, CB, D]

    w1t = np.empty((128, 2, 4, 128), np.float32)
    for kt in range(2):
        for mt in range(4):
            blk = w1b[0, 128 * mt:128 * (mt + 1), 128 * kt:128 * (kt + 1)]
            w1t[:, kt, mt, :] = blk.T
    w2t = np.empty((128, 4, 2, 128), np.float32)
    for kt in range(4):
        for mt in range(2):
            blk = w2b[L - 1, 128 * mt:128 * (mt + 1), 128 * kt:128 * (kt + 1)]
            w2t[:, kt, mt, :] = blk.T

    # fused products Wf[i] = W1[i+1] @ W2[i]  [D, D], integer entries
    wf = np.empty((L - 1, 128, 4, 4, 128), np.float32)
    for i in range(L - 1):
        prod = w1b[i + 1] @ w2b[i]  # [D, D]
        for kt in range(4):
            for mt in range(4):
                blk = prod[128 * mt:128 * (mt + 1), 128 * kt:128 * (kt + 1)]
                wf[i, :, kt, mt, :] = blk.T

    eye = np.eye(128, dtype=np.float32)
    wdg = np.empty((L, 128, 3, 4, 128), np.float32)
    for i in range(L):
        for j in range(3):
            for ct in range(4):
                wdg[i, :, j, ct, :] = eye * wdb[i, ct * 128:(ct + 1) * 128, j]

    # b1c[i] = b1[i] + W1[i] @ b2[i-1]  (conv2 bias folded across the
    # fused boundary; layer 0 keeps plain b1)
    b1c = np.array(b1, np.float32).copy()
    for i in range(1, L):
        b1c[i] += w1b[i] @ np.asarray(b2[i - 1], np.float32)

    wsa = wdb.sum(-1)
    ws12 = wdb[:, :, 1] + wdb[:, :, 2]
    ws2 = wdb[:, :, 2]
    vec_list = [b1c, bd, g1, be1, g2, be2, wsa, ws12, ws2,
                wdb[:, :, 0], wdb[:, :, 1], wdb[:, :, 2]]
    vecs = np.empty((128, L, NVEC, 4), np.float32)
    for v, arr in enumerate(vec_list):
        vecs[:, :, v, :] = np.asarray(arr).reshape(L, 4, 128).transpose(2, 0, 1)
    b2v = np.asarray(b2[L - 1]).reshape(2, 128).T.astype(np.float32)
    return (w1t.astype(bf), wf.astype(bf), w2t.astype(bf), wdg.astype(bf),
            vecs, b2v)


def kernel(**inputs):
    from concourse.bass_utils import run_bass_kernel_spmd
    import ml_dtypes

    inputs = {k: np.asarray(v, dtype=np.float32) for k, v in inputs.items()}
    x = inputs["x"]
    x_bf = x.astype(ml_dtypes.bfloat16)
    w1t, wf, w2t, wdg, vecs, b2v = _prep_inputs(**inputs)

    key = "nc"
    if key not in _cache:
        _cache[key] = _build(inputs["a1"], inputs["a2"])
    nc = _cache[key]

    in_maps = []
    for i in range(NCORES):
        in_maps.append({
            "xin": np.ascontiguousarray(x_bf[BLOC * i:BLOC * (i + 1)]),
            "w1t": w1t, "wf": wf, "w2t": w2t, "wdg": wdg, "vecs": vecs,
            "b2v": b2v,
        })
    import os
    trace = bool(int(os.environ.get("BASS_KERNEL_TRACE", "0")))
    res = run_bass_kernel_spmd(
        nc, in_maps, core_ids=list(range(NCORES)), trace=trace,
    )
    _cache["last_results"] = res
    out = np.empty((B, CB, T), np.float32)
    for i in range(NCORES):
        out[BLOC * i:BLOC * (i + 1)] = res.results[i]["out"]
    return out


# revision 11
# speedup vs baseline: 1.2089x; 1.2089x over previous
"""Trainium2 Bass kernel for BitwiseTasNetBlock (v3).

Data-parallel over batch: 8 cores x 2 batch items, activations bf16.

Structure per layer (Y1(i) already in ybuf[i%2], produced by the fused
F(i-1) matmul or by the layer-0 input conv A0):
  - dconv run-ahead: a few PE diagonal-matmul units drained raw
    (Act.Copy) so the PE works through the stats1 collective
  - sync1: AllReduce of (sum, sumsq) -> BN1 scale s1 / shift t1
  - B: depthwise dilated conv per unit on PE (diag matmuls + fused
    BN1+PReLU ACT drain) or DVE (ts+stt+stt chain with s1 folded into
    tap scalars + ACT PReLU), writing P2 in place of Y1 (reverse
    chunk-pair order)
  - sync2: AllReduce -> BN2
  - F (i<3): fused conv2(i)+conv1(i+1): lhsT = (W1(i+1)@W2(i)) *
    diag(s2) (host-precomputed integer product, exact in bf16; s2
    column-scale applied on chip), bias folded into the drain bias
    b1p(i+1) = b1(i+1) + W1(i+1)@b2(i) [host] + Wf@t2 [tiny matmul].
    Drains (ACT PReLU) write Y1(i+1) to the other ybuf.
    C (i==3): w2*s2 matmuls + bias3 + residual (DVE) -> DMA out f32.

Stats use no bn_stats: every ACT drain emits sum(y) via accum_out for
free; sum(y^2) is one extra op per unit (DVE stt-accum or ACT Square
with accum, split to balance engines).
"""
import sys

sys.path.insert(0, "/opt/trn_rl_repo")
import numpy as np

L, CB, D, KTAP = 4, 256, 512, 3
B, T = 16, 4096
EPS = 1e-5
NCORES = 8
BLOC = B // NCORES
CHUNK = 512
NCT = T // CHUNK
PAIR = 2 * CHUNK
NPAIR = NCT // 2
PAD = 16
DOFF = PAD
YCOLS = PAD + T
NGLOB = float(B * T)

# chunk pairs in reverse time order: (6,7),(4,5),(2,3),(0,1) as (c0,c1)
PAIRS = [(NCT - 2 * p - 2, NCT - 2 * p - 1) for p in range(NPAIR)]

# --- tuning knobs ---
# dconv units are (b, pair_idx, ct); DVE-mode units per layer by (b, pi):
DVE_PI = {0: [(1, 0), (1, 1)],
          1: [(1, 0), (1, 1)],
          2: [(1, 0), (1, 1)],
          3: [(1, 0), (1, 1)]}
# PE-mode units drained raw before sync1 (fixed up after):
DEFER_PI = [(0, 0), (0, 1)]
# sum(y) placement: ACT accum (cheap 287ns read) or DVE ts-accum; split
# to balance engines. sum(y^2) always DVE stt-accum.


def _sy_on_act(pi, sub):
    return True


(V_B1C, V_BD, V_G1, V_BE1, V_G2, V_BE2, V_WSA, V_WS12, V_WS2,
 V_WD0, V_WD1, V_WD2) = range(12)
NVEC = 12

_cache = {}


def _build(a1_vals, a2_vals):
    import concourse.bass as bass
    import concourse.tile as tile
    from concourse import bacc, mybir

    f32 = mybir.dt.float32
    bf16 = mybir.dt.bfloat16
    Alu = mybir.AluOpType
    Act = mybir.ActivationFunctionType

    nc = bacc.Bacc(None, target_bir_lowering=False, debug=False, num_devices=NCORES)

    xin_d = nc.dram_tensor("xin", [BLOC, CB, T], bf16, kind="ExternalInput")
    w1t_d = nc.dram_tensor("w1t", [128, 2, 4, 128], bf16, kind="ExternalInput")
    wf_d = nc.dram_tensor("wf", [L - 1, 128, 4, 4, 128], bf16, kind="ExternalInput")
    w2t_d = nc.dram_tensor("w2t", [128, 4, 2, 128], bf16, kind="ExternalInput")
    wdg_d = nc.dram_tensor("wdg", [L, 128, 3, 4, 128], bf16, kind="ExternalInput")
    vecs_d = nc.dram_tensor("vecs", [128, L, NVEC, 4], f32, kind="ExternalInput")
    b2_d = nc.dram_tensor("b2v", [128, 2], f32, kind="ExternalInput")
    out_d = nc.dram_tensor("out", [BLOC, CB, T], f32, kind="ExternalOutput")

    with tile.TileContext(nc) as tc:
        with (
            tc.tile_pool(name="ybufp", bufs=1) as ybufp,
            tc.tile_pool(name="constp", bufs=1) as constp,
            tc.tile_pool(name="wp", bufs=2) as wp,
            tc.tile_pool(name="xstg", bufs=3) as xstgp,
            tc.tile_pool(name="zbp", bufs=2) as zbp,
            tc.tile_pool(name="sqp", bufs=3) as sqp,
            tc.tile_pool(name="rstg", bufs=3) as rstgp,
            tc.tile_pool(name="ostg", bufs=2) as ostgp,
            tc.tile_pool(name="vec", bufs=8) as vecp,
            tc.tile_pool(name="ps", bufs=4, space="PSUM") as psp,
            tc.tile_pool(name="dram", bufs=4, space="DRAM") as dramp,
        ):
            # ping-pong activation buffers: ybuf[s][ct]
            ybuf = [
                [ybufp.tile([128, BLOC, YCOLS], bf16, tag=f"ybuf{s}{ct}",
                            name=f"ybuf{s}{ct}") for ct in range(4)]
                for s in range(2)
            ]

            vecs_sb = constp.tile([128, L, NVEC, 4], f32)
            b2_sb = constp.tile([128, 2], f32)
            nc.scalar.dma_start(vecs_sb[:], vecs_d[:])
            nc.scalar.dma_start(b2_sb[:], b2_d[:])

            w1sb = constp.tile([128, 2, 4, 128], bf16, tag="w1sb")
            w2sb = constp.tile([128, 4, 2, 128], bf16, tag="w2sb")
            wfsb = [wp.tile([128, 4, 4, 128], bf16, tag="wfsb", name=f"wfsb{i}")
                    for i in range(2)]
            wdsb = [wp.tile([128, 3, 4, 128], bf16, tag="wdsb", name=f"wdsb{i}")
                    for i in range(2)]
            nc.scalar.dma_start(w1sb[:], w1t_d[:])
            nc.scalar.dma_start(wdsb[0][:], wdg_d[0])
            nc.scalar.dma_start(wfsb[0][:], wf_d[0])

            # startup alignment dummy AllReduce
            aln_in = dramp.tile([128, 1], f32, tag="alnin")
            aln_out = dramp.tile([128, 1], f32, tag="alnout")
            alnsb = constp.tile([128, 1], f32, tag="alnsb")
            nc.vector.memset(alnsb[:], 0.0)
            nc.sync.dma_start(aln_in[:], alnsb[:])
            nc.gpsimd.collective_compute(
                "AllReduce", Alu.add,
                replica_groups=[list(range(NCORES))],
                ins=[aln_in[:].opt()], outs=[aln_out[:].opt()],
            )

            for s in range(2):
                for ct in range(4):
                    for b in range(BLOC):
                        nc.vector.memset(ybuf[s][ct][:, b, 0:PAD], 0.0)
            epsc = constp.tile([128, 1], f32, tag="epsc")
            nc.vector.memset(epsc[:], EPS)

            # per-layer drain bias (b1 + folded conv2 bias of prev layer)
            b1p = [vecp.tile([128, 4], f32, tag="b1p", name=f"b1p{i}")
                   for i in range(L)]
            nc.vector.tensor_copy(b1p[0][:], vecs_sb[:, 0, V_B1C, :])

            # stats accumulators per layer: [128, 4(sub), 8(b*4+pi)]
            st1 = []
            st2 = []
            for i in range(L):
                st1.append((
                    vecp.tile([128, 4, 8], f32, tag="asy", name=f"asy{i}"),
                    vecp.tile([128, 4, 8], f32, tag="asq", name=f"asq{i}"),
                ))
                st2.append((
                    vecp.tile([128, 4, 8], f32, tag="bsy", name=f"bsy{i}"),
                    vecp.tile([128, 4, 8], f32, tag="bsq", name=f"bsq{i}"),
                ))

            def sq_accum(y_sl, sy_sl, sq_sl, pi, sub):
                """sum(y) (if sy_sl given) and sum(y^2) for one [1024]
                unit on the vector engine."""
                if sy_sl is not None:
                    scr0 = sqp.tile([128, PAIR], bf16, tag="sy_scr")
                    nc.vector.tensor_scalar(
                        scr0[:], y_sl, 1.0, 0.0, op0=Alu.mult, op1=Alu.add,
                        accum_out=sy_sl)
                scr = sqp.tile([128, PAIR], bf16, tag="sqv")
                nc.vector.scalar_tensor_tensor(
                    scr[:], y_sl, 1.0, y_sl, op0=Alu.mult, op1=Alu.mult,
                    accum_out=sq_sl)

            def stat_sync(i, st, g_idx, be_idx):
                """Reduce accumulators, AllReduce, produce s4/t4 [128,4]."""
                sy, sq = st
                csb = vecp.tile([128, 8], f32, tag="csb")
                nc.vector.tensor_reduce(
                    csb[:, 0:4], sy[:], axis=mybir.AxisListType.X, op=Alu.add)
                nc.vector.tensor_reduce(
                    csb[:, 4:8], sq[:], axis=mybir.AxisListType.X, op=Alu.add)
                cin = dramp.tile([128, 8], f32, tag="cin")
                cout = dramp.tile([128, 8], f32, tag="cout")
                nc.sync.dma_start(cin[:], csb[:])
                nc.gpsimd.collective_compute(
                    "AllReduce", Alu.add,
                    replica_groups=[list(range(NCORES))],
                    ins=[cin[:].opt()], outs=[cout[:].opt()],
                )
                gsb = vecp.tile([128, 8], f32, tag="gsb")
                nc.sync.dma_start(gsb[:], cout[:])
                mean4 = vecp.tile([128, 4], f32, tag="mean4")
                nc.vector.tensor_scalar(
                    mean4[:], gsb[:, 0:4], 1.0 / NGLOB, None, op0=Alu.mult)
                var4 = vecp.tile([128, 4], f32, tag="var4")
                nc.vector.tensor_scalar(
                    var4[:], gsb[:, 4:8], 1.0 / NGLOB, None, op0=Alu.mult)
                m24 = vecp.tile([128, 4], f32, tag="m24")
                nc.vector.tensor_mul(m24[:], mean4[:], mean4[:])
                nc.vector.tensor_sub(var4[:], var4[:], m24[:])
                std4 = vecp.tile([128, 4], f32, tag="std4")
                nc.scalar.activation(std4[:], var4[:], Act.Sqrt, bias=epsc[:],
                                     scale=1.0)
                rstd4 = vecp.tile([128, 4], f32, tag="rstd4")
                nc.vector.reciprocal(rstd4[:], std4[:])
                s4 = vecp.tile([128, 4], f32, tag="s4")
                nc.vector.tensor_mul(s4[:], rstd4[:], vecs_sb[:, i, g_idx, :])
                t4 = vecp.tile([128, 4], f32, tag="t4")
                nc.vector.tensor_mul(t4[:], mean4[:], s4[:])
                nc.vector.tensor_sub(t4[:], vecs_sb[:, i, be_idx, :], t4[:])
                return s4, t4

            # ---- A0: layer-0 input conv ----
            a1_0 = float(a1_vals[0])
            for b in range(BLOC):
                for pi, (c0, c1) in enumerate(PAIRS):
                    xst = []
                    for c in (c0, c1):
                        xs = xstgp.tile([128, PAIR], bf16, tag="xs",
                                        name=f"xs_{b}_{c}")
                        for kt in range(2):
                            eng = nc.sync if kt == 0 else nc.scalar
                            eng.dma_start(
                                xs[:, kt * CHUNK:(kt + 1) * CHUNK],
                                xin_d[b, 128 * kt:128 * (kt + 1),
                                      CHUNK * c:CHUNK * (c + 1)])
                        xst.append(xs)
                    for mt in range(4):
                        ps = psp.tile([128, PAIR], f32, tag="ps")
                        for kt in range(2):
                            for h in range(2):
                                nc.tensor.matmul(
                                    ps[:, h * CHUNK:(h + 1) * CHUNK],
                                    w1sb[:, kt, mt, :],
                                    xst[h][:, kt * CHUNK:(kt + 1) * CHUNK],
                                    start=(kt == 0), stop=(kt == 1))
                        ysl = ybuf[0][mt][:, b, DOFF + CHUNK * c0:
                                          DOFF + CHUNK * c0 + PAIR]
                        sy_sl = st1[0][0][:, mt, b * 4 + pi:b * 4 + pi + 1]
                        on_act = _sy_on_act(pi, mt)
                        nc.scalar.activation(
                            ysl, ps[:], Act.Prelu, bias=b1p[0][:, mt:mt + 1],
                            scale=1.0, alpha=a1_0,
                            accum_out=sy_sl if on_act else None)
                        sq_accum(ysl, None if on_act else sy_sl,
                                 st1[0][1][:, mt, b * 4 + pi:b * 4 + pi + 1],
                                 pi, mt)

            # ---- layer loop ----
            for i in range(L):
                dil = 2 ** i
                a2i = float(a2_vals[i])
                wsl = i % 2
                ybi = ybuf[i % 2]
                wdr = wdsb[wsl]
                if i + 1 < L:
                    nsl = (i + 1) % 2
                    nc.scalar.dma_start(wdsb[nsl][:], wdg_d[i + 1])
                    if i + 1 <= L - 2:
                        nc.scalar.dma_start(wfsb[nsl][:], wf_d[i + 1])
                if i == L - 2:
                    nc.scalar.dma_start(w2sb[:], w2t_d[:])

                def emit_dconv_mm(b, c0, c1, ct, _dil=dil, _wdr=wdr, _ybi=ybi,
                                  _i=i):
                    ps = psp.tile([128, PAIR], f32, tag="ps",
                                  name=f"psd_{_i}_{b}_{c0}_{ct}")
                    for j in range(KTAP):
                        for h, c in ((0, c0), (1, c1)):
                            base = DOFF + CHUNK * c
                            off = base - (2 - j) * _dil
                            nc.tensor.matmul(
                                ps[:, h * CHUNK:(h + 1) * CHUNK],
                                _wdr[:, j, ct, :],
                                _ybi[ct][:, b, off:off + CHUNK],
                                start=(j == 0), stop=(j == KTAP - 1))
                    return ps

                # run-ahead PE units, raw drains (no s1 dependency)
                deferred = []
                for (b, pi) in DEFER_PI:
                    c0, c1 = PAIRS[pi]
                    for ct in range(4):
                        ps = emit_dconv_mm(b, c0, c1, ct)
                        dst = ybi[ct][:, b, DOFF + CHUNK * c0:
                                      DOFF + CHUNK * c0 + PAIR]
                        nc.scalar.activation(dst, ps[:], Act.Copy)
                        deferred.append((b, pi, c0, ct))

                # ---- sync1 ----
                s1, t1 = stat_sync(i, st1[i], V_G1, V_BE1)
                b2a4 = vecp.tile([128, 4], f32, tag="b2a4")
                b2b4 = vecp.tile([128, 4], f32, tag="b2b4")
                b2c4 = vecp.tile([128, 4], f32, tag="b2c4")
                for dst4, widx in ((b2a4, V_WSA), (b2b4, V_WS12), (b2c4, V_WS2)):
                    nc.vector.tensor_mul(dst4[:], t1[:], vecs_sb[:, i, widx, :])
                    nc.vector.tensor_add(dst4[:], dst4[:], vecs_sb[:, i, V_BD, :])
                s1w = []
                for j in range(KTAP):
                    sw = vecp.tile([128, 4], f32, tag=f"s1w{j}", name=f"s1w{j}_{i}")
                    nc.vector.tensor_mul(sw[:], s1[:], vecs_sb[:, i, V_WD0 + j, :])
                    s1w.append(sw)

                # deferred fixups: in-place PReLU with BN1 scale/shift + stats
                for (b, pi, c0, ct) in deferred:
                    dst = ybi[ct][:, b, DOFF + CHUNK * c0:DOFF + CHUNK * c0 + PAIR]
                    sy_sl = st2[i][0][:, ct, b * 4 + pi:b * 4 + pi + 1]
                    on_act = _sy_on_act(pi, ct)
                    nc.scalar.activation(
                        dst, dst, Act.Prelu, bias=b2a4[:, ct:ct + 1],
                        scale=s1[:, ct:ct + 1], alpha=a2i,
                        accum_out=sy_sl if on_act else None)
                    sq_accum(dst, None if on_act else sy_sl,
                             st2[i][1][:, ct, b * 4 + pi:b * 4 + pi + 1],
                             pi, ct)

                # ---- B: remaining dconv units ----
                for b in range(BLOC):
                    for pi, (c0, c1) in enumerate(PAIRS):
                        if (b, pi) in DEFER_PI:
                            continue
                        for ct in range(4):
                            dst = ybi[ct][:, b, DOFF + CHUNK * c0:
                                          DOFF + CHUNK * c0 + PAIR]
                            sy_sl = st2[i][0][:, ct, b * 4 + pi:b * 4 + pi + 1]
                            on_act = _sy_on_act(pi, ct)
                            acc_sl = sy_sl if on_act else None
                            if (b, pi) in DVE_PI[i]:
                                zb = zbp.tile([128, PAIR], bf16, tag="zb")
                                base = DOFF + CHUNK * c0
                                nc.vector.tensor_scalar(
                                    zb[:], ybi[ct][:, b, base - 2 * dil:
                                                   base - 2 * dil + PAIR],
                                    s1w[0][:, ct:ct + 1], b2a4[:, ct:ct + 1],
                                    op0=Alu.mult, op1=Alu.add)
                                nc.vector.scalar_tensor_tensor(
                                    zb[:], ybi[ct][:, b, base - dil:
                                                   base - dil + PAIR],
                                    s1w[1][:, ct:ct + 1], zb[:],
                                    op0=Alu.mult, op1=Alu.add)
                                nc.vector.scalar_tensor_tensor(
                                    zb[:], ybi[ct][:, b, base:base + PAIR],
                                    s1w[2][:, ct:ct + 1], zb[:],
                                    op0=Alu.mult, op1=Alu.add)
                                nc.scalar.activation(dst, zb[:], Act.Prelu,
                                                     alpha=a2i, accum_out=acc_sl)
                            else:
                                ps = emit_dconv_mm(b, c0, c1, ct)
                                nc.scalar.activation(
                                    dst, ps[:], Act.Prelu,
                                    bias=b2a4[:, ct:ct + 1],
                                    scale=s1[:, ct:ct + 1], alpha=a2i,
                                    accum_out=acc_sl)
                                if c0 == 0:
                                    po = DOFF
                                    nc.scalar.activation(
                                        ybi[ct][:, b, po:po + dil], ps[:, 0:dil],
                                        Act.Prelu, bias=b2c4[:, ct:ct + 1],
                                        scale=s1[:, ct:ct + 1], alpha=a2i)
                                    nc.scalar.activation(
                                        ybi[ct][:, b, po + dil:po + 2 * dil],
                                        ps[:, dil:2 * dil], Act.Prelu,
                                        bias=b2b4[:, ct:ct + 1],
                                        scale=s1[:, ct:ct + 1], alpha=a2i)
                            sq_accum(dst, None if on_act else sy_sl,
                                     st2[i][1][:, ct, b * 4 + pi:b * 4 + pi + 1],
                                     pi, ct)
                # note: the c0==0 boundary rewrites (2*dil of 65536 cols per
                # channel) slightly perturb sum(y)/sum(y^2) vs exact; the
                # relative stats error is ~dil/65536 -- far below bf16 noise.

                # ---- sync2 + weight folding ----
                s2, t2 = stat_sync(i, st2[i], V_G2, V_BE2)
                if i < L - 1:
                    a1n = float(a1_vals[i + 1])
                    wfr = wfsb[wsl]
                    wfs = wp.tile([128, 4, 4, 128], bf16, tag="wfs")
                    for kt in range(4):
                        nc.vector.tensor_scalar(
                            wfs[:, kt, :, :], wfr[:, kt, :, :],
                            s2[:, kt:kt + 1], None, op0=Alu.mult)
                    # b1p[i+1] = b1c[i+1] + Wf_raw @ t2
                    t2bf = vecp.tile([128, 4, 2], bf16, tag="t2bf")
                    nc.vector.tensor_copy(t2bf[:, :, 0], t2[:])
                    nc.vector.tensor_copy(t2bf[:, :, 1], t2[:])
                    psf = psp.tile([128, PAIR], f32, tag="ps", name=f"psf_{i}")
                    for mt in range(4):
                        for kt in range(4):
                            nc.tensor.matmul(
                                psf[:, 2 * mt:2 * mt + 2],
                                wfr[:, kt, mt, :],
                                t2bf[:, kt, :],
                                start=(kt == 0), stop=(kt == 3))
                        nc.scalar.activation(
                            b1p[i + 1][:, mt:mt + 1], psf[:, 2 * mt:2 * mt + 1],
                            Act.Identity,
                            bias=vecs_sb[:, i + 1, V_B1C, mt:mt + 1], scale=1.0)
                    # ---- F: fused conv2(i) + conv1(i+1) ----
                    ybn = ybuf[(i + 1) % 2]
                    for b in range(BLOC):
                        for pi, (c0, c1) in enumerate(PAIRS):
                            po0 = DOFF + CHUNK * c0
                            for mt in range(4):
                                ps = psp.tile([128, PAIR], f32, tag="ps")
                                for kt in range(4):
                                    for h in range(2):
                                        nc.tensor.matmul(
                                            ps[:, h * CHUNK:(h + 1) * CHUNK],
                                            wfs[:, kt, mt, :],
                                            ybi[kt][:, b, po0 + h * CHUNK:
                                                    po0 + (h + 1) * CHUNK],
                                            start=(kt == 0), stop=(kt == 3))
                                ysl = ybn[mt][:, b, po0:po0 + PAIR]
                                sy_sl = st1[i + 1][0][
                                    :, mt, b * 4 + pi:b * 4 + pi + 1]
                                on_act = _sy_on_act(pi, mt)
                                nc.scalar.activation(
                                    ysl, ps[:], Act.Prelu,
                                    bias=b1p[i + 1][:, mt:mt + 1], scale=1.0,
                                    alpha=a1n,
                                    accum_out=sy_sl if on_act else None)
                                sq_accum(ysl, None if on_act else sy_sl,
                                         st1[i + 1][1][:, mt,
                                                       b * 4 + pi:b * 4 + pi + 1],
                                         pi, mt)
                else:
                    # ---- C (last layer): w2*s2 + bias3 + residual ----
                    w2r = wp.tile([128, 4, 2, 128], bf16, tag="w2r")
                    for kt in range(4):
                        nc.vector.tensor_scalar(
                            w2r[:, kt, :, :], w2sb[:, kt, :, :],
                            s2[:, kt:kt + 1], None, op0=Alu.mult)
                    rec4 = vecp.tile([128, 4], f32, tag="rec4")
                    nc.vector.reciprocal(rec4[:], s2[:])
                    r24 = vecp.tile([128, 4, 2], bf16, tag="r24")
                    nc.vector.tensor_mul(r24[:, :, 0], t2[:], rec4[:])
                    nc.vector.tensor_mul(r24[:, :, 1], t2[:], rec4[:])
                    psb = psp.tile([128, PAIR], f32, tag="ps", name="psb3")
                    b3f = vecp.tile([128, 2], f32, tag="b3f")
                    for mt in range(2):
                        for kt in range(4):
                            nc.tensor.matmul(
                                psb[:, 2 * mt:2 * mt + 2],
                                w2r[:, kt, mt, :],
                                r24[:, kt, :],
                                start=(kt == 0), stop=(kt == 3))
                        nc.scalar.activation(
                            b3f[:, mt:mt + 1], psb[:, 2 * mt:2 * mt + 1],
                            Act.Identity, bias=b2_sb[:, mt:mt + 1], scale=1.0)
                    for b in range(BLOC):
                        for c in range(NCT - 1, -1, -1):
                            po = DOFF + CHUNK * c
                            rt = rstgp.tile([128, PAIR], bf16, tag="rt")
                            for mt in range(2):
                                nc.scalar.dma_start(
                                    rt[:, mt * CHUNK:(mt + 1) * CHUNK],
                                    xin_d[b, 128 * mt:128 * (mt + 1),
                                          CHUNK * c:CHUNK * (c + 1)])
                            ps = psp.tile([128, PAIR], f32, tag="ps")
                            for mt in range(2):
                                for kt in range(4):
                                    nc.tensor.matmul(
                                        ps[:, mt * CHUNK:(mt + 1) * CHUNK],
                                        w2r[:, kt, mt, :],
                                        ybi[kt][:, b, po:po + CHUNK],
                                        start=(kt == 0), stop=(kt == 3))
                            ot = ostgp.tile([128, PAIR], f32, tag="ot")
                            for mt in range(2):
                                nc.vector.scalar_tensor_tensor(
                                    ot[:, mt * CHUNK:(mt + 1) * CHUNK],
                                    ps[:, mt * CHUNK:(mt + 1) * CHUNK],
                                    b3f[:, mt:mt + 1],
                                    rt[:, mt * CHUNK:(mt + 1) * CHUNK],
                                    op0=Alu.add, op1=Alu.add)
                                eng = nc.sync if c % 2 == 0 else nc.scalar
                                eng.dma_start(
                                    out_d[b, 128 * mt:128 * (mt + 1),
                                          CHUNK * c:CHUNK * (c + 1)],
                                    ot[:, mt * CHUNK:(mt + 1) * CHUNK])

    nc.compile()
    return nc


def _prep_inputs(x, w1, b1, a1, g1, be1, wd, bd, a2, g2, be2, w2, b2):
    """Host-side packing. Binarized weights; fused W1(i+1)@W2(i) products
    (integer-valued, exact in bf16)."""
    import ml_dtypes

    bf = ml_dtypes.bfloat16
    w1b = np.sign(w1[..., 0]).astype(np.float32)  # [L, D, CB]
    wdb = np.sign(wd[..., 0, :]) if wd.ndim == 4 else np.sign(wd[:, :, 0, :])
    wdb = wdb.astype(np.float32)  # [L, D, K]
    w2b = np.sign(w2[..., 0]).astype(np.float32)  # [L, CB, D]

    w1t = np.empty((128, 2, 4, 128), np.float32)
    for kt in range(2):
        for mt in range(4):
            blk = w1b[0, 128 * mt:128 * (mt + 1), 128 * kt:128 * (kt + 1)]
            w1t[:, kt, mt, :] = blk.T
    w2t = np.empty((128, 4, 2, 128), np.float32)
    for kt in range(4):
        for mt in range(2):
            blk = w2b[L - 1, 128 * mt:128 * (mt + 1), 128 * kt:128 * (kt + 1)]
            w2t[:, kt, mt, :] = blk.T

    # fused products Wf[i] = W1[i+1] @ W2[i]  [D, D], integer entries
    wf = np.empty((L - 1, 128, 4, 4, 128), np.float32)
    for i in range(L - 1):
        prod = w1b[i + 1] @ w2b[i]  # [D, D]
        for kt in range(4):
            for mt in range(4):
                blk = prod[128 * mt:128 * (mt + 1), 128 * kt:128 * (kt + 1)]
                wf[i, :, kt, mt, :] = blk.T

    eye = np.eye(128, dtype=np.float32)
    wdg = np.empty((L, 128, 3, 4, 128), np.float32)
    for i in range(L):
        for j in range(3):
            for ct in range(4):
                wdg[i, :, j, ct, :] = eye * wdb[i, ct * 128:(ct + 1) * 128, j]

    # b1c[i] = b1[i] + W1[i] @ b2[i-1]  (conv2 bias folded across the
    # fused boundary; layer 0 keeps plain b1)
    b1c = np.array(b1, np.float32).copy()
    for i in range(1, L):
        b1c[i] += w1b[i] @ np.asarray(b2[i - 1], np.float32)

    wsa = wdb.sum(-1)
    ws12 = wdb[:, :, 1] + wdb[:, :, 2]
    ws2 = wdb[:, :, 2]
    vec_list = [b1c, bd, g1, be1, g2, be2, wsa, ws12, ws2,
                wdb[:, :, 0], wdb[:, :, 1], wdb[:, :, 2]]
    vecs = np.empty((128, L, NVEC, 4), np.float32)
    for v, arr in enumerate(vec_list):
        vecs[:, :, v, :] = np.asarray(arr).reshape(L, 4, 128).transpose(2, 0, 1)
    b2v = np.asarray(b2[L - 1]).reshape(2, 128).T.astype(np.float32)
    return (w1t.astype(bf), wf.astype(bf), w2t.astype(bf), wdg.astype(bf),
            vecs, b2v)


def kernel(**inputs):
    from concourse.bass_utils import run_bass_kernel_spmd
    import ml_dtypes

    inputs = {k: np.asarray(v, dtype=np.float32) for k, v in inputs.items()}
    x = inputs["x"]
    x_bf = x.astype(ml_dtypes.bfloat16)
    w1t, wf, w2t, wdg, vecs, b2v = _prep_inputs(**inputs)

    key = "nc"
    if key not in _cache:
        _cache[key] = _build(inputs["a1"], inputs["a2"])
    nc = _cache[key]

    in_maps = []
    for i in range(NCORES):
        in_maps.append({
            "xin": np.ascontiguousarray(x_bf[BLOC * i:BLOC * (i + 1)]),
            "w1t": w1t, "wf": wf, "w2t": w2t, "wdg": wdg, "vecs": vecs,
            "b2v": b2v,
        })
    import os
    trace = bool(int(os.environ.get("BASS_KERNEL_TRACE", "0")))
    res = run_bass_kernel_spmd(
        nc, in_maps, core_ids=list(range(NCORES)), trace=trace,
    )
    _cache["last_results"] = res
    out = np.empty((B, CB, T), np.float32)
    for i in range(NCORES):
        out[BLOC * i:BLOC * (i + 1)] = res.results[i]["out"]
    return out


# revision 14
# speedup vs baseline: 1.2230x; 1.0117x over previous
"""Trainium2 Bass kernel for BitwiseTasNetBlock (v3).

Data-parallel over batch: 8 cores x 2 batch items, activations bf16.

Structure per layer (Y1(i) already in ybuf[i%2], produced by the fused
F(i-1) matmul or by the layer-0 input conv A0):
  - dconv run-ahead: a few PE diagonal-matmul units drained raw
    (Act.Copy) so the PE works through the stats1 collective
  - sync1: AllReduce of (sum, sumsq) -> BN1 scale s1 / shift t1
  - B: depthwise dilated conv per unit on PE (diag matmuls + fused
    BN1+PReLU ACT drain) or DVE (ts+stt+stt chain with s1 folded into
    tap scalars + ACT PReLU), writing P2 in place of Y1 (reverse
    chunk-pair order)
  - sync2: AllReduce -> BN2
  - F (i<3): fused conv2(i)+conv1(i+1): lhsT = (W1(i+1)@W2(i)) *
    diag(s2) (host-precomputed integer product, exact in bf16; s2
    column-scale applied on chip), bias folded into the drain bias
    b1p(i+1) = b1(i+1) + W1(i+1)@b2(i) [host] + Wf@t2 [tiny matmul].
    Drains (ACT PReLU) write Y1(i+1) to the other ybuf.
    C (i==3): w2*s2 matmuls + bias3 + residual (DVE) -> DMA out f32.

Stats use no bn_stats: every ACT drain emits sum(y) via accum_out for
free; sum(y^2) is one extra op per unit (DVE stt-accum or ACT Square
with accum, split to balance engines).
"""
import sys

sys.path.insert(0, "/opt/trn_rl_repo")
import numpy as np

L, CB, D, KTAP = 4, 256, 512, 3
B, T = 16, 4096
EPS = 1e-5
NCORES = 8
BLOC = B // NCORES
CHUNK = 512
NCT = T // CHUNK
PAIR = 2 * CHUNK
NPAIR = NCT // 2
PAD = 16
DOFF = PAD
YCOLS = PAD + T
NGLOB = float(B * T)

# chunk pairs in reverse time order: (6,7),(4,5),(2,3),(0,1) as (c0,c1)
PAIRS = [(NCT - 2 * p - 2, NCT - 2 * p - 1) for p in range(NPAIR)]

# --- tuning knobs ---
# dconv units are (b, pair_idx, ct); DVE-mode units per layer by (b, pi):
DVE_PI = {0: [(1, 0), (1, 1)],
          1: [(1, 0), (1, 1)],
          2: [(1, 0), (1, 1)],
          3: [(1, 0), (1, 1)]}
# PE-mode units drained raw before sync1 (fixed up after):
DEFER_PI = [(0, 0), (0, 1)]
# sum(y) placement: ACT accum (cheap 287ns read) or DVE ts-accum; split
# to balance engines. sum(y^2) always DVE stt-accum.


def _sy_on_act(pi, sub):
    return True


(V_B1C, V_BD, V_G1, V_BE1, V_G2, V_BE2, V_WSA, V_WS12, V_WS2,
 V_WD0, V_WD1, V_WD2, V_RG1) = range(13)
NVEC = 13

_cache = {}


def _build(a1_vals, a2_vals):
    import concourse.bass as bass
    import concourse.tile as tile
    from concourse import bacc, mybir

    f32 = mybir.dt.float32
    bf16 = mybir.dt.bfloat16
    Alu = mybir.AluOpType
    Act = mybir.ActivationFunctionType

    nc = bacc.Bacc(None, target_bir_lowering=False, debug=False, num_devices=NCORES)

    xin_d = nc.dram_tensor("xin", [BLOC, CB, T], bf16, kind="ExternalInput")
    w1t_d = nc.dram_tensor("w1t", [128, 2, 4, 128], bf16, kind="ExternalInput")
    wf_d = nc.dram_tensor("wf", [L - 1, 128, 4, 4, 128], bf16, kind="ExternalInput")
    w2t_d = nc.dram_tensor("w2t", [128, 4, 2, 128], bf16, kind="ExternalInput")
    wdg_d = nc.dram_tensor("wdg", [L, 128, 3, 4, 128], bf16, kind="ExternalInput")
    vecs_d = nc.dram_tensor("vecs", [128, L, NVEC, 4], f32, kind="ExternalInput")
    b2_d = nc.dram_tensor("b2v", [128, 2], f32, kind="ExternalInput")
    out_d = nc.dram_tensor("out", [BLOC, CB, T], f32, kind="ExternalOutput")

    with tile.TileContext(nc) as tc:
        with (
            tc.tile_pool(name="ybufp", bufs=1) as ybufp,
            tc.tile_pool(name="constp", bufs=1) as constp,
            tc.tile_pool(name="wp", bufs=2) as wp,
            tc.tile_pool(name="xstg", bufs=3) as xstgp,
            tc.tile_pool(name="zbp", bufs=2) as zbp,
            tc.tile_pool(name="sqp", bufs=3) as sqp,
            tc.tile_pool(name="rstg", bufs=3) as rstgp,
            tc.tile_pool(name="ostg", bufs=2) as ostgp,
            tc.tile_pool(name="vec", bufs=8) as vecp,
            tc.tile_pool(name="ps", bufs=4, space="PSUM") as psp,
            tc.tile_pool(name="dram", bufs=4, space="DRAM") as dramp,
        ):
            # ping-pong activation buffers: ybuf[s][ct]
            ybuf = [
                [ybufp.tile([128, BLOC, YCOLS], bf16, tag=f"ybuf{s}{ct}",
                            name=f"ybuf{s}{ct}") for ct in range(4)]
                for s in range(2)
            ]

            vecs_sb = constp.tile([128, L, NVEC, 4], f32)
            b2_sb = constp.tile([128, 2], f32)
            nc.scalar.dma_start(vecs_sb[:], vecs_d[:])
            nc.scalar.dma_start(b2_sb[:], b2_d[:])

            w1sb = constp.tile([128, 2, 4, 128], bf16, tag="w1sb")
            w2sb = constp.tile([128, 4, 2, 128], bf16, tag="w2sb")
            wfsb = [wp.tile([128, 4, 4, 128], bf16, tag="wfsb", name=f"wfsb{i}")
                    for i in range(2)]
            wdsb = [wp.tile([128, 3, 4, 128], bf16, tag="wdsb", name=f"wdsb{i}")
                    for i in range(2)]
            nc.scalar.dma_start(w1sb[:], w1t_d[:])
            nc.scalar.dma_start(wdsb[0][:], wdg_d[0])
            nc.scalar.dma_start(wfsb[0][:], wf_d[0])

            # startup alignment dummy AllReduce
            aln_in = dramp.tile([128, 1], f32, tag="alnin")
            aln_out = dramp.tile([128, 1], f32, tag="alnout")
            alnsb = constp.tile([128, 1], f32, tag="alnsb")
            nc.vector.memset(alnsb[:], 0.0)
            nc.sync.dma_start(aln_in[:], alnsb[:])
            nc.gpsimd.collective_compute(
                "AllReduce", Alu.add,
                replica_groups=[list(range(NCORES))],
                ins=[aln_in[:].opt()], outs=[aln_out[:].opt()],
            )

            for s in range(2):
                for ct in range(4):
                    for b in range(BLOC):
                        nc.vector.memset(ybuf[s][ct][:, b, 0:PAD], 0.0)
            epsc = constp.tile([128, 1], f32, tag="epsc")
            nc.vector.memset(epsc[:], EPS)
            ones16 = constp.tile([128, PAD], bf16, tag="ones16")
            nc.vector.memset(ones16[:], 1.0)

            # per-layer drain bias (b1 + folded conv2 bias of prev layer)
            b1p = [vecp.tile([128, 4], f32, tag="b1p", name=f"b1p{i}")
                   for i in range(L)]
            nc.vector.tensor_copy(b1p[0][:], vecs_sb[:, 0, V_B1C, :])

            # stats accumulators per layer: [128, 4(sub), 8(b*4+pi)]
            st1 = []
            st2 = []
            for i in range(L):
                st1.append((
                    vecp.tile([128, 4, 8], f32, tag="asy", name=f"asy{i}"),
                    vecp.tile([128, 4, 4], f32, tag="asq", name=f"asq{i}"),
                ))
                st2.append((
                    vecp.tile([128, 4, 8], f32, tag="bsy", name=f"bsy{i}"),
                    vecp.tile([128, 4, 4], f32, tag="bsq", name=f"bsq{i}"),
                ))

            def sq_quad(yb_sub, b, q, sq_sl):
                """sum(y^2) over one [2048] quad (pairs 2q..2q+1) of one
                channel-tile row on the vector engine."""
                lo = DOFF + (2048 if q == 0 else 0)
                y_sl = yb_sub[:, b, lo:lo + 2048]
                scr = sqp.tile([128, 2048], bf16, tag="sqv")
                nc.vector.scalar_tensor_tensor(
                    scr[:], y_sl, 1.0, y_sl, op0=Alu.mult, op1=Alu.mult,
                    accum_out=sq_sl)

            def stat_sync(i, st, g_idx, be_idx):
                """Reduce accumulators, AllReduce, produce s4/t4 [128,4]."""
                sy, sq = st
                csb = vecp.tile([128, 8], f32, tag="csb")
                nc.vector.tensor_reduce(
                    csb[:, 0:4], sy[:], axis=mybir.AxisListType.X, op=Alu.add)
                nc.vector.tensor_reduce(
                    csb[:, 4:8], sq[:], axis=mybir.AxisListType.X, op=Alu.add)
                cin = dramp.tile([128, 8], f32, tag="cin")
                cout = dramp.tile([128, 8], f32, tag="cout")
                nc.sync.dma_start(cin[:], csb[:])
                nc.gpsimd.collective_compute(
                    "AllReduce", Alu.add,
                    replica_groups=[list(range(NCORES))],
                    ins=[cin[:].opt()], outs=[cout[:].opt()],
                )
                gsb = vecp.tile([128, 8], f32, tag="gsb")
                nc.sync.dma_start(gsb[:], cout[:])
                mean4 = vecp.tile([128, 4], f32, tag="mean4")
                nc.vector.tensor_scalar(
                    mean4[:], gsb[:, 0:4], 1.0 / NGLOB, None, op0=Alu.mult)
                var4 = vecp.tile([128, 4], f32, tag="var4")
                nc.vector.tensor_scalar(
                    var4[:], gsb[:, 4:8], 1.0 / NGLOB, None, op0=Alu.mult)
                m24 = vecp.tile([128, 4], f32, tag="m24")
                nc.vector.tensor_mul(m24[:], mean4[:], mean4[:])
                nc.vector.tensor_sub(var4[:], var4[:], m24[:])
                std4 = vecp.tile([128, 4], f32, tag="std4")
                nc.scalar.activation(std4[:], var4[:], Act.Sqrt, bias=epsc[:],
                                     scale=1.0)
                rstd4 = vecp.tile([128, 4], f32, tag="rstd4")
                nc.vector.reciprocal(rstd4[:], std4[:])
                s4 = vecp.tile([128, 4], f32, tag="s4")
                nc.vector.tensor_mul(s4[:], rstd4[:], vecs_sb[:, i, g_idx, :])
                t4 = vecp.tile([128, 4], f32, tag="t4")
                nc.vector.tensor_mul(t4[:], mean4[:], s4[:])
                nc.vector.tensor_sub(t4[:], vecs_sb[:, i, be_idx, :], t4[:])
                return s4, t4, std4

            # ---- A0: layer-0 input conv ----
            a1_0 = float(a1_vals[0])
            for b in range(BLOC):
                for pi, (c0, c1) in enumerate(PAIRS):
                    xst = []
                    for c in (c0, c1):
                        xs = xstgp.tile([128, PAIR], bf16, tag="xs",
                                        name=f"xs_{b}_{c}")
                        for kt in range(2):
                            eng = nc.sync if kt == 0 else nc.scalar
                            eng.dma_start(
                                xs[:, kt * CHUNK:(kt + 1) * CHUNK],
                                xin_d[b, 128 * kt:128 * (kt + 1),
                                      CHUNK * c:CHUNK * (c + 1)])
                        xst.append(xs)
                    for mt in range(4):
                        ps = psp.tile([128, PAIR], f32, tag="ps")
                        for kt in range(2):
                            for h in range(2):
                                nc.tensor.matmul(
                                    ps[:, h * CHUNK:(h + 1) * CHUNK],
                                    w1sb[:, kt, mt, :],
                                    xst[h][:, kt * CHUNK:(kt + 1) * CHUNK],
                                    start=(kt == 0), stop=(kt == 1))
                        ysl = ybuf[0][mt][:, b, DOFF + CHUNK * c0:
                                          DOFF + CHUNK * c0 + PAIR]
                        nc.scalar.activation(
                            ysl, ps[:], Act.Prelu, bias=b1p[0][:, mt:mt + 1],
                            scale=1.0, alpha=a1_0,
                            accum_out=st1[0][0][:, mt, b * 4 + pi:b * 4 + pi + 1])
                    if pi % 2 == 1:
                        q = pi // 2
                        for mt in range(4):
                            sq_quad(ybuf[0][mt], b, q,
                                    st1[0][1][:, mt, b * 2 + q:b * 2 + q + 1])

            # ---- layer loop ----
            for i in range(L):
                dil = 2 ** i
                a2i = float(a2_vals[i])
                wsl = i % 2
                ybi = ybuf[i % 2]
                wdr = wdsb[wsl]
                if i + 1 < L:
                    nsl = (i + 1) % 2
                    nc.scalar.dma_start(wdsb[nsl][:], wdg_d[i + 1])
                    if i + 1 <= L - 2:
                        nc.scalar.dma_start(wfsb[nsl][:], wf_d[i + 1])
                if i == L - 2:
                    nc.scalar.dma_start(w2sb[:], w2t_d[:])

                def emit_dconv_mm(b, c0, c1, ct, _dil=dil, _wdr=wdr, _ybi=ybi,
                                  _i=i):
                    ps = psp.tile([128, PAIR], f32, tag="ps",
                                  name=f"psd_{_i}_{b}_{c0}_{ct}")
                    for j in range(KTAP):
                        for h, c in ((0, c0), (1, c1)):
                            base = DOFF + CHUNK * c
                            off = base - (2 - j) * _dil
                            nc.tensor.matmul(
                                ps[:, h * CHUNK:(h + 1) * CHUNK],
                                _wdr[:, j, ct, :],
                                _ybi[ct][:, b, off:off + CHUNK],
                                start=(j == 0), stop=(j == KTAP - 1))
                    return ps

                # run-ahead PE units, raw drains (no s1 dependency)
                deferred = []
                for (b, pi) in DEFER_PI:
                    c0, c1 = PAIRS[pi]
                    for ct in range(4):
                        ps = emit_dconv_mm(b, c0, c1, ct)
                        dst = ybi[ct][:, b, DOFF + CHUNK * c0:
                                      DOFF + CHUNK * c0 + PAIR]
                        nc.scalar.activation(dst, ps[:], Act.Copy)
                        deferred.append((b, pi, c0, ct))

                # ---- sync1 ----
                s1, t1, std1 = stat_sync(i, st1[i], V_G1, V_BE1)
                b2a4 = vecp.tile([128, 4], f32, tag="b2a4")
                nc.vector.tensor_mul(b2a4[:], t1[:], vecs_sb[:, i, V_WSA, :])
                nc.vector.tensor_add(b2a4[:], b2a4[:], vecs_sb[:, i, V_BD, :])
                # causal-pad values: PAD = t1/s1 = t1*std/g1, so the missing
                # taps at the causal edge contribute exactly t1*wd_j after
                # the BN1 fold -- no boundary-column fixups needed.
                t1s = vecp.tile([128, 4], f32, tag="t1s")
                nc.vector.tensor_scalar(t1s[:], t1[:], -1.0, None, op0=Alu.mult)
                nc.vector.tensor_mul(t1s[:], t1s[:], std1[:])
                nc.vector.tensor_mul(t1s[:], t1s[:], vecs_sb[:, i, V_RG1, :])
                for ct in range(4):
                    for b in range(BLOC):
                        nc.vector.tensor_scalar(
                            ybi[ct][:, b, 0:PAD], ones16[:],
                            t1s[:, ct:ct + 1], None, op0=Alu.mult)
                s1w = []
                for j in range(KTAP):
                    sw = vecp.tile([128, 4], f32, tag=f"s1w{j}", name=f"s1w{j}_{i}")
                    nc.vector.tensor_mul(sw[:], s1[:], vecs_sb[:, i, V_WD0 + j, :])
                    s1w.append(sw)

                # deferred fixups: in-place PReLU with BN1 scale/shift + stats
                for (b, pi, c0, ct) in deferred:
                    dst = ybi[ct][:, b, DOFF + CHUNK * c0:DOFF + CHUNK * c0 + PAIR]
                    nc.scalar.activation(
                        dst, dst, Act.Prelu, bias=b2a4[:, ct:ct + 1],
                        scale=s1[:, ct:ct + 1], alpha=a2i,
                        accum_out=st2[i][0][:, ct, b * 4 + pi:b * 4 + pi + 1])
                for ct in range(4):
                    sq_quad(ybi[ct], 0, 0, st2[i][1][:, ct, 0:1])

                # ---- B: remaining dconv units ----
                for b in range(BLOC):
                    for pi, (c0, c1) in enumerate(PAIRS):
                        if (b, pi) in DEFER_PI:
                            continue
                        for ct in range(4):
                            dst = ybi[ct][:, b, DOFF + CHUNK * c0:
                                          DOFF + CHUNK * c0 + PAIR]
                            acc_sl = st2[i][0][:, ct, b * 4 + pi:b * 4 + pi + 1]
                            if (b, pi) in DVE_PI[i]:
                                zb = zbp.tile([128, PAIR], bf16, tag="zb")
                                base = DOFF + CHUNK * c0
                                nc.vector.tensor_scalar(
                                    zb[:], ybi[ct][:, b, base - 2 * dil:
                                                   base - 2 * dil + PAIR],
                                    s1w[0][:, ct:ct + 1], b2a4[:, ct:ct + 1],
                                    op0=Alu.mult, op1=Alu.add)
                                nc.vector.scalar_tensor_tensor(
                                    zb[:], ybi[ct][:, b, base - dil:
                                                   base - dil + PAIR],
                                    s1w[1][:, ct:ct + 1], zb[:],
                                    op0=Alu.mult, op1=Alu.add)
                                nc.vector.scalar_tensor_tensor(
                                    zb[:], ybi[ct][:, b, base:base + PAIR],
                                    s1w[2][:, ct:ct + 1], zb[:],
                                    op0=Alu.mult, op1=Alu.add)
                                nc.scalar.activation(dst, zb[:], Act.Prelu,
                                                     alpha=a2i, accum_out=acc_sl)
                            else:
                                ps = emit_dconv_mm(b, c0, c1, ct)
                                nc.scalar.activation(
                                    dst, ps[:], Act.Prelu,
                                    bias=b2a4[:, ct:ct + 1],
                                    scale=s1[:, ct:ct + 1], alpha=a2i,
                                    accum_out=acc_sl)
                        if pi % 2 == 1 and not (b == 0 and pi == 1):
                            q = pi // 2
                            for ct in range(4):
                                sq_quad(ybi[ct], b, q,
                                        st2[i][1][:, ct, b * 2 + q:b * 2 + q + 1])
                # note: the c0==0 boundary rewrites (2*dil of 65536 cols per
                # channel) slightly perturb sum(y)/sum(y^2) vs exact; the
                # relative stats error is ~dil/65536 -- far below bf16 noise.

                # ---- sync2 + weight folding ----
                s2, t2, _std2 = stat_sync(i, st2[i], V_G2, V_BE2)
                if i < L - 1:
                    a1n = float(a1_vals[i + 1])
                    wfr = wfsb[wsl]
                    wfs = wp.tile([128, 4, 4, 128], bf16, tag="wfs")
                    for kt in range(4):
                        nc.vector.tensor_scalar(
                            wfs[:, kt, :, :], wfr[:, kt, :, :],
                            s2[:, kt:kt + 1], None, op0=Alu.mult)
                    # b1p[i+1] = b1c[i+1] + Wf_raw @ t2
                    t2bf = vecp.tile([128, 4, 2], bf16, tag="t2bf")
                    nc.vector.tensor_copy(t2bf[:, :, 0], t2[:])
                    nc.vector.tensor_copy(t2bf[:, :, 1], t2[:])
                    psf = psp.tile([128, PAIR], f32, tag="ps", name=f"psf_{i}")
                    for mt in range(4):
                        for kt in range(4):
                            nc.tensor.matmul(
                                psf[:, 2 * mt:2 * mt + 2],
                                wfr[:, kt, mt, :],
                                t2bf[:, kt, :],
                                start=(kt == 0), stop=(kt == 3))
                        nc.scalar.activation(
                            b1p[i + 1][:, mt:mt + 1], psf[:, 2 * mt:2 * mt + 1],
                            Act.Identity,
                            bias=vecs_sb[:, i + 1, V_B1C, mt:mt + 1], scale=1.0)
                    # ---- F: fused conv2(i) + conv1(i+1) ----
                    ybn = ybuf[(i + 1) % 2]
                    for b in range(BLOC):
                        for pi, (c0, c1) in enumerate(PAIRS):
                            po0 = DOFF + CHUNK * c0
                            for mt in range(4):
                                ps = psp.tile([128, PAIR], f32, tag="ps")
                                for kt in range(4):
                                    for h in range(2):
                                        nc.tensor.matmul(
                                            ps[:, h * CHUNK:(h + 1) * CHUNK],
                                            wfs[:, kt, mt, :],
                                            ybi[kt][:, b, po0 + h * CHUNK:
                                                    po0 + (h + 1) * CHUNK],
                                            start=(kt == 0), stop=(kt == 3))
                                ysl = ybn[mt][:, b, po0:po0 + PAIR]
                                nc.scalar.activation(
                                    ysl, ps[:], Act.Prelu,
                                    bias=b1p[i + 1][:, mt:mt + 1], scale=1.0,
                                    alpha=a1n,
                                    accum_out=st1[i + 1][0][
                                        :, mt, b * 4 + pi:b * 4 + pi + 1])
                            if pi % 2 == 1:
                                q = pi // 2
                                for mt in range(4):
                                    sq_quad(ybn[mt], b, q,
                                            st1[i + 1][1][:, mt,
                                                          b * 2 + q:b * 2 + q + 1])
                else:
                    # ---- C (last layer): w2*s2 + bias3 + residual ----
                    w2r = wp.tile([128, 4, 2, 128], bf16, tag="w2r")
                    for kt in range(4):
                        nc.vector.tensor_scalar(
                            w2r[:, kt, :, :], w2sb[:, kt, :, :],
                            s2[:, kt:kt + 1], None, op0=Alu.mult)
                    rec4 = vecp.tile([128, 4], f32, tag="rec4")
                    nc.vector.reciprocal(rec4[:], s2[:])
                    r24 = vecp.tile([128, 4, 2], bf16, tag="r24")
                    nc.vector.tensor_mul(r24[:, :, 0], t2[:], rec4[:])
                    nc.vector.tensor_mul(r24[:, :, 1], t2[:], rec4[:])
                    psb = psp.tile([128, PAIR], f32, tag="ps", name="psb3")
                    b3f = vecp.tile([128, 2], f32, tag="b3f")
                    for mt in range(2):
                        for kt in range(4):
                            nc.tensor.matmul(
                                psb[:, 2 * mt:2 * mt + 2],
                                w2r[:, kt, mt, :],
                                r24[:, kt, :],
                                start=(kt == 0), stop=(kt == 3))
                        nc.scalar.activation(
                            b3f[:, mt:mt + 1], psb[:, 2 * mt:2 * mt + 1],
                            Act.Identity, bias=b2_sb[:, mt:mt + 1], scale=1.0)
                    for b in range(BLOC):
                        for c in range(NCT - 1, -1, -1):
                            po = DOFF + CHUNK * c
                            rt = rstgp.tile([128, PAIR], bf16, tag="rt")
                            for mt in range(2):
                                nc.scalar.dma_start(
                                    rt[:, mt * CHUNK:(mt + 1) * CHUNK],
                                    xin_d[b, 128 * mt:128 * (mt + 1),
                                          CHUNK * c:CHUNK * (c + 1)])
                            ps = psp.tile([128, PAIR], f32, tag="ps")
                            for mt in range(2):
                                for kt in range(4):
                                    nc.tensor.matmul(
                                        ps[:, mt * CHUNK:(mt + 1) * CHUNK],
                                        w2r[:, kt, mt, :],
                                        ybi[kt][:, b, po:po + CHUNK],
                                        start=(kt == 0), stop=(kt == 3))
                            ot = ostgp.tile([128, PAIR], f32, tag="ot")
                            for mt in range(2):
                                nc.vector.scalar_tensor_tensor(
                                    ot[:, mt * CHUNK:(mt + 1) * CHUNK],
                                    ps[:, mt * CHUNK:(mt + 1) * CHUNK],
                                    b3f[:, mt:mt + 1],
                                    rt[:, mt * CHUNK:(mt + 1) * CHUNK],
                                    op0=Alu.add, op1=Alu.add)
                                eng = nc.sync if c % 2 == 0 else nc.scalar
                                eng.dma_start(
                                    out_d[b, 128 * mt:128 * (mt + 1),
                                          CHUNK * c:CHUNK * (c + 1)],
                                    ot[:, mt * CHUNK:(mt + 1) * CHUNK])

    nc.compile()
    return nc


def _prep_inputs(x, w1, b1, a1, g1, be1, wd, bd, a2, g2, be2, w2, b2):
    """Host-side packing. Binarized weights; fused W1(i+1)@W2(i) products
    (integer-valued, exact in bf16)."""
    import ml_dtypes

    bf = ml_dtypes.bfloat16
    w1b = np.sign(w1[..., 0]).astype(np.float32)  # [L, D, CB]
    wdb = np.sign(wd[..., 0, :]) if wd.ndim == 4 else np.sign(wd[:, :, 0, :])
    wdb = wdb.astype(np.float32)  # [L, D, K]
    w2b = np.sign(w2[..., 0]).astype(np.float32)  # [L, CB, D]

    w1t = np.empty((128, 2, 4, 128), np.float32)
    for kt in range(2):
        for mt in range(4):
            blk = w1b[0, 128 * mt:128 * (mt + 1), 128 * kt:128 * (kt + 1)]
            w1t[:, kt, mt, :] = blk.T
    w2t = np.empty((128, 4, 2, 128), np.float32)
    for kt in range(4):
        for mt in range(2):
            blk = w2b[L - 1, 128 * mt:128 * (mt + 1), 128 * kt:128 * (kt + 1)]
            w2t[:, kt, mt, :] = blk.T

    # fused products Wf[i] = W1[i+1] @ W2[i]  [D, D], integer entries
    wf = np.empty((L - 1, 128, 4, 4, 128), np.float32)
    for i in range(L - 1):
        prod = w1b[i + 1] @ w2b[i]  # [D, D]
        for kt in range(4):
            for mt in range(4):
                blk = prod[128 * mt:128 * (mt + 1), 128 * kt:128 * (kt + 1)]
                wf[i, :, kt, mt, :] = blk.T

    eye = np.eye(128, dtype=np.float32)
    wdg = np.empty((L, 128, 3, 4, 128), np.float32)
    for i in range(L):
        for j in range(3):
            for ct in range(4):
                wdg[i, :, j, ct, :] = eye * wdb[i, ct * 128:(ct + 1) * 128, j]

    # b1c[i] = b1[i] + W1[i] @ b2[i-1]  (conv2 bias folded across the
    # fused boundary; layer 0 keeps plain b1)
    b1c = np.array(b1, np.float32).copy()
    for i in range(1, L):
        b1c[i] += w1b[i] @ np.asarray(b2[i - 1], np.float32)

    wsa = wdb.sum(-1)
    ws12 = wdb[:, :, 1] + wdb[:, :, 2]
    ws2 = wdb[:, :, 2]
    g1a = np.asarray(g1, np.float32)
    rg1 = np.where(g1a == 0, 0.0, 1.0 / np.where(g1a == 0, 1.0, g1a))
    vec_list = [b1c, bd, g1, be1, g2, be2, wsa, ws12, ws2,
                wdb[:, :, 0], wdb[:, :, 1], wdb[:, :, 2], rg1]
    vecs = np.empty((128, L, NVEC, 4), np.float32)
    for v, arr in enumerate(vec_list):
        vecs[:, :, v, :] = np.asarray(arr).reshape(L, 4, 128).transpose(2, 0, 1)
    b2v = np.asarray(b2[L - 1]).reshape(2, 128).T.astype(np.float32)
    return (w1t.astype(bf), wf.astype(bf), w2t.astype(bf), wdg.astype(bf),
            vecs, b2v)


def kernel(**inputs):
    from concourse.bass_utils import run_bass_kernel_spmd
    import ml_dtypes

    inputs = {k: np.asarray(v, dtype=np.float32) for k, v in inputs.items()}
    x = inputs["x"]
    x_bf = x.astype(ml_dtypes.bfloat16)
    w1t, wf, w2t, wdg, vecs, b2v = _prep_inputs(**inputs)

    key = "nc"
    if key not in _cache:
        _cache[key] = _build(inputs["a1"], inputs["a2"])
    nc = _cache[key]

    in_maps = []
    for i in range(NCORES):
        in_maps.append({
            "xin": np.ascontiguousarray(x_bf[BLOC * i:BLOC * (i + 1)]),
            "w1t": w1t, "wf": wf, "w2t": w2t, "wdg": wdg, "vecs": vecs,
            "b2v": b2v,
        })
    import os
    trace = bool(int(os.environ.get("BASS_KERNEL_TRACE", "0")))
    res = run_bass_kernel_spmd(
        nc, in_maps, core_ids=list(range(NCORES)), trace=trace,
    )
    _cache["last_results"] = res
    out = np.empty((B, CB, T), np.float32)
    for i in range(NCORES):
        out[BLOC * i:BLOC * (i + 1)] = res.results[i]["out"]
    return out
